# revision 28
# baseline (speedup 1.0000x reference)
"""Trainium2 kernel for SparseLinear + bias + SELU (nn_AEEncoder).

Reference computation:
    y[b, o] = selu( sum_{e: out_idx[e]==o} weight[e] * x[b, in_idx[e]] + bias[o] )
with B=512, IN_F=20000, OUT_F=1000, NNZ=500000.

Strategy
--------
The edge list arrives as concrete numpy arrays at call time, so the sparse
weights are densified on the host into W[IN_F, OUT_F] (duplicate edges
accumulate). The device kernel is then a dense matmul y = x @ W + bias
followed by SELU, executed in bf16 (f32 PSUM accumulation).

Sharding: a 2-way batch x 4-way output-column grid over the 8 NeuronCores.
Each core computes a full [256, 250] block of the output independently —
no cross-core collectives (collectives pay a large all-core sync barrier
under this runner). The bias is folded into the matmul as one extra
contraction row (x^T gets a row of ones, W gets the bias row), so the
on-chip epilogue is only the SELU.

Per-core: x^T shard [20096, 256] bf16 + W shard [20096, 250] bf16 are
DMA-streamed to SBUF in k-tile groups (partition-major DRAM layout so
every DMA is long contiguous runs); 314 accumulating matmuls
(2 M-tiles x 157 K-tiles, N=250) run concurrently with the DMA stream;
SELU is computed straight out of PSUM and the [256, 250] f32 block is
DMA'd out. The host assembles the 2x4 grid into the full [512, 1000].
"""

import numpy as np
import ml_dtypes

import concourse.bass as bass
import concourse.mybir as mybir
import concourse.tile as tile
from concourse import bacc
from concourse.bass_utils import run_bass_kernel_spmd

B, IN_F, OUT_F = 512, 20000, 1000
NCORES = 8
BS, OS = 2, 4          # batch split x out-column split (BS*OS == NCORES)
BSH = B // BS          # 256 batch rows per core
OSH = OUT_F // OS      # 250 output columns per core
KPAD = 20096           # padded contraction dim: 157 k-tiles of 128
KT = KPAD // 128       # 157 k-tiles (row IN_F==20000 carries the bias)
# k-tiles per DMA group: moderate first group -> PE starts soon; ramp-down
# -> short compute tail after the final transfer. Sums to KT.
GROUPS = [6, 14, 20, 20, 20, 20, 20, 17, 10, 6, 4]
TAIL_M = 16            # last k-tiles emitted m-major so epilogue(m=0)
                       # overlaps the final matmuls of m=1
MT = BSH // 128        # 2 M-tiles per core

SELU_SCALE = 1.0507009873554805
SELU_ALPHA = 1.6732632423543772

_compiled = None


def _build():
    nc = bacc.Bacc("TRN2", target_bir_lowering=False, debug=False,
                   num_devices=NCORES)
    # partition-major layouts: row p holds that partition's whole k-stream,
    # so each DMA is 128 fully-contiguous runs (no small-packet penalty)
    xt_d = nc.dram_tensor("xt", [128, KT * BSH], mybir.dt.bfloat16,
                          kind="ExternalInput")
    w_d = nc.dram_tensor("w", [128, KT * OSH], mybir.dt.bfloat16,
                         kind="ExternalInput")
    out_d = nc.dram_tensor("out", [BSH, OSH], mybir.dt.float32,
                           kind="ExternalOutput")

    with tile.TileContext(nc) as tc:
        with (
            tc.tile_pool(name="sb", bufs=1) as sb,
            tc.tile_pool(name="ps", bufs=1, space="PSUM") as ps,
        ):
            xt_sb = sb.tile([128, KT * BSH], mybir.dt.bfloat16)
            w_sb = sb.tile([128, KT * OSH], mybir.dt.bfloat16)
            assert sum(GROUPS) == KT
            g0 = 0
            for gsz in GROUPS:
                g1 = g0 + gsz
                nc.sync.dma_start(
                    xt_sb[:, g0 * BSH:g1 * BSH],
                    xt_d[:, g0 * BSH:g1 * BSH],
                )
                nc.scalar.dma_start(
                    w_sb[:, g0 * OSH:g1 * OSH],
                    w_d[:, g0 * OSH:g1 * OSH],
                )
                g0 = g1

            accs = [
                ps.tile([128, OSH], mybir.dt.float32,
                        name=f"acc{m}", tag=f"acc{m}")
                for m in range(MT)
            ]
            def mm(k, m):
                nc.tensor.matmul(
                    accs[m][:],
                    xt_sb[:, k * BSH + m * 128: k * BSH + (m + 1) * 128],
                    w_sb[:, k * OSH:(k + 1) * OSH],
                    start=(k == 0),
                    stop=(k == KT - 1),
                )

            for k in range(KT - TAIL_M):
                for m in range(MT):
                    mm(k, m)
            for m in range(MT):
                for k in range(KT - TAIL_M, KT):
                    mm(k, m)

            import math
            lam_al = SELU_SCALE * SELU_ALPHA
            lnb = sb.tile([128, 1], mybir.dt.float32)
            nc.vector.memset(lnb[:], math.log(lam_al))
            NCHUNK = 4
            CW = OSH // NCHUNK  # 62/63-col chunks: DVE/ACT chains pipeline
            for m in range(MT):
                # selu(v) = lam*relu(v) + lam*al*(exp(min(v,0)) - 1)
                #         = (lam*al*exp(min(v,0)) - lam*al) + lam*relu(v)
                rl = sb.tile([128, OSH], mybir.dt.float32,
                             name=f"rl{m}", tag=f"rl{m}")
                mn = sb.tile([128, OSH], mybir.dt.float32,
                             name=f"mn{m}", tag=f"mn{m}")
                ex = sb.tile([128, OSH], mybir.dt.float32,
                             name=f"ex{m}", tag=f"ex{m}")
                oo = sb.tile([128, OSH], mybir.dt.float32,
                             name=f"oo{m}", tag=f"oo{m}")
                for h in range(NCHUNK):
                    cs = slice(h * CW, (h + 1) * CW if h < NCHUNK - 1
                               else OSH)
                    nc.scalar.activation(rl[:, cs], accs[m][:, cs],
                                         mybir.ActivationFunctionType.Relu,
                                         scale=SELU_SCALE)
                    nc.vector.tensor_scalar_min(mn[:, cs], accs[m][:, cs],
                                                0.0)
                    # exp(mn + ln(lam*al)) == lam*al*exp(mn)
                    nc.scalar.activation(ex[:, cs], mn[:, cs],
                                         mybir.ActivationFunctionType.Exp,
                                         bias=lnb[:])
                    nc.vector.scalar_tensor_tensor(
                        oo[:, cs], ex[:, cs], -lam_al, rl[:, cs],
                        mybir.AluOpType.add, mybir.AluOpType.add)
                nc.sync.dma_start(out_d[m * 128:(m + 1) * 128, :], oo[:])

    nc.compile()
    return nc


def _build_raw():
    """Hand-synchronized build (no TileContext): cuts Tile's entry/exit
    barriers and starts the DMA streams immediately."""
    import math
    from contextlib import ExitStack

    lam_al = SELU_SCALE * SELU_ALPHA
    nc = bacc.Bacc("TRN2", target_bir_lowering=False, debug=False,
                   num_devices=NCORES)
    xt_d = nc.dram_tensor("xt", [128, KT * BSH], mybir.dt.bfloat16,
                          kind="ExternalInput")
    w_d = nc.dram_tensor("w", [128, KT * OSH], mybir.dt.bfloat16,
                         kind="ExternalInput")
    out_d = nc.dram_tensor("out", [BSH, OSH], mybir.dt.float32,
                           kind="ExternalOutput")

    bounds = []
    g0 = 0
    for gsz in GROUPS:
        bounds.append((g0, g0 + gsz))
        g0 += gsz
    assert g0 == KT
    NG = len(bounds)

    with ExitStack() as ctx:
        xt_sb = ctx.enter_context(
            nc.sbuf_tensor("xt_sb", [128, KT * BSH], mybir.dt.bfloat16))
        w_sb = ctx.enter_context(
            nc.sbuf_tensor("w_sb", [128, KT * OSH], mybir.dt.bfloat16))
        # one PSUM bank per accumulator (512 f32 = full bank) so the two
        # accumulators never share a bank (PE-write/DVE-read hazard)
        accs = [ctx.enter_context(
            nc.psum_tensor(f"acc{m}", [128, 512], mybir.dt.float32))
            for m in range(MT)]
        mn = [ctx.enter_context(
            nc.sbuf_tensor(f"mn{m}", [128, OSH], mybir.dt.float32))
            for m in range(MT)]
        ex = [ctx.enter_context(
            nc.sbuf_tensor(f"ex{m}", [128, OSH], mybir.dt.float32))
            for m in range(MT)]
        rl = [ctx.enter_context(
            nc.sbuf_tensor(f"rl{m}", [128, OSH], mybir.dt.float32))
            for m in range(MT)]
        oo = [ctx.enter_context(
            nc.sbuf_tensor(f"oo{m}", [128, OSH], mybir.dt.float32))
            for m in range(MT)]
        lnb = ctx.enter_context(
            nc.sbuf_tensor("lnb", [128, 1], mybir.dt.float32))

        xsems = [ctx.enter_context(nc.semaphore(name=f"xsem{g}"))
                 for g in range(NG)]
        wsems = [ctx.enter_context(nc.semaphore(name=f"wsem{g}"))
                 for g in range(NG)]
        pe_sem = ctx.enter_context(nc.semaphore(name="pe_sem"))
        dve_sem = ctx.enter_context(nc.semaphore(name="dve_sem"))
        act_sem = ctx.enter_context(nc.semaphore(name="act_sem"))
        o_sem = ctx.enter_context(nc.semaphore(name="o_sem"))
        od_sem = ctx.enter_context(nc.semaphore(name="od_sem"))
        block = ctx.enter_context(nc.Block())

        @block.sync
        def _(sync):
            for g, (a, b) in enumerate(bounds):
                sync.dma_start(
                    xt_sb[:, a * BSH:b * BSH],
                    xt_d[:, a * BSH:b * BSH]).then_inc(xsems[g], 16)
            for m in range(MT):
                sync.wait_ge(o_sem, m + 1)
                sync.dma_start(
                    out_d[m * 128:(m + 1) * 128, :],
                    oo[m][:]).then_inc(od_sem, 16)
            sync.wait_ge(od_sem, 32)

        # NOTE: the ScalarEngine must not read PSUM on this runner (hard
        # device fault, found by bisection) — all PSUM reads go via DVE.
        @block.scalar
        def _(scalar):
            for g, (a, b) in enumerate(bounds):
                scalar.dma_start(
                    w_sb[:, a * OSH:b * OSH],
                    w_d[:, a * OSH:b * OSH]).then_inc(wsems[g], 16)
            for m in range(MT):
                scalar.wait_ge(dve_sem, m + 2)  # memset(1) + mn[m]
                scalar.activation(ex[m][:], mn[m][:],
                                  mybir.ActivationFunctionType.Exp,
                                  bias=lnb[:]).then_inc(act_sem, 1)

        @block.vector
        def _(vector):
            vector.memset(lnb[:], math.log(lam_al)).then_inc(dve_sem, 1)
            for m in range(MT):
                vector.wait_ge(pe_sem, m + 1)
                # rl = max(lam*acc, 0) == lam*relu(acc); consumed only by
                # this engine's own stt below -> no semaphore needed
                vector.tensor_scalar(rl[m][:], accs[m][:, :OSH],
                                     SELU_SCALE, 0.0,
                                     mybir.AluOpType.mult,
                                     mybir.AluOpType.max)
                vector.tensor_scalar_min(
                    mn[m][:], accs[m][:, :OSH], 0.0).then_inc(dve_sem, 1)
                vector.wait_ge(act_sem, m + 1)
                vector.scalar_tensor_tensor(
                    oo[m][:], ex[m][:], -lam_al, rl[m][:],
                    mybir.AluOpType.add,
                    mybir.AluOpType.add).then_inc(o_sem, 1)

        @block.tensor
        def _(tensor):
            def mm(k, m):
                return tensor.matmul(
                    accs[m][:, :OSH],
                    xt_sb[:, k * BSH + m * 128: k * BSH + (m + 1) * 128],
                    w_sb[:, k * OSH:(k + 1) * OSH],
                    start=(k == 0),
                    stop=(k == KT - 1),
                )

            for g, (a, b) in enumerate(bounds):
                tensor.wait_ge(xsems[g], 16)
                tensor.wait_ge(wsems[g], 16)
                for k in range(a, min(b, KT - TAIL_M)):
                    for m in range(MT):
                        mm(k, m)
            for m in range(MT):
                for k in range(KT - TAIL_M, KT):
                    inst = mm(k, m)
                inst.then_inc(pe_sem, 1)

    nc.compile()
    return nc


def _prepare_in_maps(x, weight, bias, out_idx, in_idx):
    x = np.asarray(x, dtype=np.float32)
    weight = np.asarray(weight, dtype=np.float32)
    bias = np.asarray(bias, dtype=np.float32)
    oi = np.asarray(out_idx).astype(np.int64)
    ii = np.asarray(in_idx).astype(np.int64)

    # densify the edge list; duplicate (i, o) pairs accumulate
    W = np.bincount(ii * OUT_F + oi, weights=weight.astype(np.float64),
                    minlength=IN_F * OUT_F).astype(np.float32)
    W = W.reshape(IN_F, OUT_F)

    Wp = np.zeros((KPAD, OUT_F), dtype=np.float32)
    Wp[:IN_F] = W
    Wp[IN_F] = bias           # bias row, matched by the ones row in x^T
    xtp = np.zeros((KPAD, B), dtype=np.float32)
    xtp[:IN_F] = x.T
    xtp[IN_F] = 1.0

    w_bf = Wp.astype(ml_dtypes.bfloat16)
    xt_bf = xtp.astype(ml_dtypes.bfloat16)

    in_maps = []
    for c in range(NCORES):
        b, o = divmod(c, OS)
        xt_shard = np.ascontiguousarray(xt_bf[:, b * BSH:(b + 1) * BSH])
        w_shard = np.ascontiguousarray(w_bf[:, o * OSH:(o + 1) * OSH])
        in_maps.append({
            # -> partition-major [128, KT*cols]
            "xt": np.ascontiguousarray(
                xt_shard.reshape(KT, 128, BSH).transpose(1, 0, 2)
            ).reshape(128, KT * BSH),
            "w": np.ascontiguousarray(
                w_shard.reshape(KT, 128, OSH).transpose(1, 0, 2)
            ).reshape(128, KT * OSH),
        })
    return in_maps


def _assemble(results):
    y = np.empty((B, OUT_F), dtype=np.float32)
    for c in range(NCORES):
        b, o = divmod(c, OS)
        y[b * BSH:(b + 1) * BSH, o * OSH:(o + 1) * OSH] = results[c]["out"]
    return y


USE_RAW = False  # hand-synchronized build vs TileContext build


def get_compiled():
    global _compiled
    if _compiled is None:
        import os
        raw = USE_RAW if os.environ.get("KERNEL_RAW") is None \
            else os.environ["KERNEL_RAW"] == "1"
        _compiled = _build_raw() if raw else _build()
    return _compiled


def kernel(x, weight, bias, out_idx, in_idx):
    in_maps = _prepare_in_maps(x, weight, bias, out_idx, in_idx)
    nc = get_compiled()
    last_err = None
    for _attempt in range(3):  # retry transient device/runtime hiccups
        try:
            res = run_bass_kernel_spmd(nc, in_maps,
                                       core_ids=list(range(NCORES)))
            return _assemble(res.results)
        except Exception as e:  # noqa: BLE001
            last_err = e
    raise last_err


# revision 33
# speedup vs baseline: 1.1138x; 1.1138x over previous
"""Trainium2 kernel for SparseLinear + bias + SELU (nn_AEEncoder).

Reference computation:
    y[b, o] = selu( sum_{e: out_idx[e]==o} weight[e] * x[b, in_idx[e]] + bias[o] )
with B=512, IN_F=20000, OUT_F=1000, NNZ=500000.

Strategy
--------
The edge list arrives as concrete numpy arrays at call time, so the sparse
weights are densified on the host into W[IN_F, OUT_F] (duplicate edges
accumulate). The device kernel is then a dense matmul y = x @ W + bias
followed by SELU, executed in bf16 (f32 PSUM accumulation).

Sharding: a 2-way batch x 4-way output-column grid over the 8 NeuronCores.
Each core computes a full [256, 250] block of the output independently —
no cross-core collectives (collectives pay a large all-core sync barrier
under this runner). The bias is folded into the matmul as one extra
contraction row (x^T gets a row of ones, W gets the bias row), so the
on-chip epilogue is only the SELU.

Per-core: x^T shard [20096, 256] bf16 + W shard [20096, 250] bf16 are
DMA-streamed to SBUF in k-tile groups (partition-major DRAM layout so
every DMA is long contiguous runs); 314 accumulating matmuls
(2 M-tiles x 157 K-tiles, N=250) run concurrently with the DMA stream;
SELU is computed straight out of PSUM and the [256, 250] f32 block is
DMA'd out. The host assembles the 2x4 grid into the full [512, 1000].
"""

import numpy as np
import ml_dtypes

import concourse.bass as bass
import concourse.mybir as mybir
import concourse.tile as tile
from concourse import bacc
from concourse.bass_utils import run_bass_kernel_spmd

B, IN_F, OUT_F = 512, 20000, 1000
NCORES = 8
BS, OS = 2, 4          # batch split x out-column split (BS*OS == NCORES)
BSH = B // BS          # 256 batch rows per core
OSH = OUT_F // OS      # 250 output columns per core
KPAD = 20096           # padded contraction dim: 157 k-tiles of 128
KT = KPAD // 128       # 157 k-tiles (row IN_F==20000 carries the bias)
# k-tiles per DMA group: moderate first group -> PE starts soon; ramp-down
# -> short compute tail after the final transfer. Sums to KT.
GROUPS = [6, 14, 20, 20, 20, 20, 20, 17, 10, 6, 4]
TAIL_M = 16            # last k-tiles emitted m-major so epilogue(m=0)
                       # overlaps the final matmuls of m=1
MT = BSH // 128        # 2 M-tiles per core
NCHUNK = 2             # epilogue column chunks

SELU_SCALE = 1.0507009873554805
SELU_ALPHA = 1.6732632423543772

_compiled = None


def _build():
    nc = bacc.Bacc("TRN2", target_bir_lowering=False, debug=False,
                   num_devices=NCORES)
    # partition-major layouts: row p holds that partition's whole k-stream,
    # so each DMA is 128 fully-contiguous runs (no small-packet penalty)
    xt_d = nc.dram_tensor("xt", [128, KT * BSH], mybir.dt.bfloat16,
                          kind="ExternalInput")
    w_d = nc.dram_tensor("w", [128, KT * OSH], mybir.dt.bfloat16,
                         kind="ExternalInput")
    # device emits bf16; the host upcasts to float32 (error budget allows it,
    # halves the output traffic and speeds the final DVE writes)
    out_d = nc.dram_tensor("out", [BSH, OSH], mybir.dt.bfloat16,
                           kind="ExternalOutput")

    with tile.TileContext(nc) as tc:
        with (
            tc.tile_pool(name="sb", bufs=1) as sb,
            tc.tile_pool(name="ps", bufs=1, space="PSUM") as ps,
        ):
            xt_sb = sb.tile([128, KT * BSH], mybir.dt.bfloat16)
            w_sb = sb.tile([128, KT * OSH], mybir.dt.bfloat16)
            assert sum(GROUPS) == KT
            g0 = 0
            for gsz in GROUPS:
                g1 = g0 + gsz
                nc.sync.dma_start(
                    xt_sb[:, g0 * BSH:g1 * BSH],
                    xt_d[:, g0 * BSH:g1 * BSH],
                )
                nc.scalar.dma_start(
                    w_sb[:, g0 * OSH:g1 * OSH],
                    w_d[:, g0 * OSH:g1 * OSH],
                )
                g0 = g1

            accs = [
                ps.tile([128, OSH], mybir.dt.float32,
                        name=f"acc{m}", tag=f"acc{m}")
                for m in range(MT)
            ]
            def mm(k, m):
                nc.tensor.matmul(
                    accs[m][:],
                    xt_sb[:, k * BSH + m * 128: k * BSH + (m + 1) * 128],
                    w_sb[:, k * OSH:(k + 1) * OSH],
                    start=(k == 0),
                    stop=(k == KT - 1),
                )

            for k in range(KT - TAIL_M):
                for m in range(MT):
                    mm(k, m)
            for m in range(MT):
                for k in range(KT - TAIL_M, KT):
                    mm(k, m)

            import math
            lam_al = SELU_SCALE * SELU_ALPHA
            lnb = sb.tile([128, 1], mybir.dt.float32)
            nc.vector.memset(lnb[:], math.log(lam_al))
            CW = OSH // NCHUNK  # 62/63-col chunks: DVE/ACT chains pipeline
            for m in range(MT):
                # selu(v) = lam*relu(v) + lam*al*(exp(min(v,0)) - 1)
                #         = (lam*al*exp(min(v,0)) - lam*al) + lam*relu(v)
                rl = sb.tile([128, OSH], mybir.dt.float32,
                             name=f"rl{m}", tag=f"rl{m}")
                mn = sb.tile([128, OSH], mybir.dt.float32,
                             name=f"mn{m}", tag=f"mn{m}")
                ex = sb.tile([128, OSH], mybir.dt.float32,
                             name=f"ex{m}", tag=f"ex{m}")
                oo = sb.tile([128, OSH], mybir.dt.bfloat16,
                             name=f"oo{m}", tag=f"oo{m}")
                for h in range(NCHUNK):
                    cs = slice(h * CW, (h + 1) * CW if h < NCHUNK - 1
                               else OSH)
                    nc.scalar.activation(rl[:, cs], accs[m][:, cs],
                                         mybir.ActivationFunctionType.Relu,
                                         scale=SELU_SCALE)
                    nc.vector.tensor_scalar_min(mn[:, cs], accs[m][:, cs],
                                                0.0)
                    # exp(mn + ln(lam*al)) == lam*al*exp(mn)
                    nc.scalar.activation(ex[:, cs], mn[:, cs],
                                         mybir.ActivationFunctionType.Exp,
                                         bias=lnb[:])
                    nc.vector.scalar_tensor_tensor(
                        oo[:, cs], ex[:, cs], -lam_al, rl[:, cs],
                        mybir.AluOpType.add, mybir.AluOpType.add)
                nc.sync.dma_start(out_d[m * 128:(m + 1) * 128, :], oo[:])

    nc.compile()
    return nc


def _build_raw():
    """Hand-synchronized build (no TileContext): cuts Tile's entry/exit
    barriers and starts the DMA streams immediately."""
    import math
    from contextlib import ExitStack

    lam_al = SELU_SCALE * SELU_ALPHA
    nc = bacc.Bacc("TRN2", target_bir_lowering=False, debug=False,
                   num_devices=NCORES)
    xt_d = nc.dram_tensor("xt", [128, KT * BSH], mybir.dt.bfloat16,
                          kind="ExternalInput")
    w_d = nc.dram_tensor("w", [128, KT * OSH], mybir.dt.bfloat16,
                         kind="ExternalInput")
    out_d = nc.dram_tensor("out", [BSH, OSH], mybir.dt.float32,
                           kind="ExternalOutput")

    bounds = []
    g0 = 0
    for gsz in GROUPS:
        bounds.append((g0, g0 + gsz))
        g0 += gsz
    assert g0 == KT
    NG = len(bounds)

    with ExitStack() as ctx:
        xt_sb = ctx.enter_context(
            nc.sbuf_tensor("xt_sb", [128, KT * BSH], mybir.dt.bfloat16))
        w_sb = ctx.enter_context(
            nc.sbuf_tensor("w_sb", [128, KT * OSH], mybir.dt.bfloat16))
        # one PSUM bank per accumulator (512 f32 = full bank) so the two
        # accumulators never share a bank (PE-write/DVE-read hazard)
        accs = [ctx.enter_context(
            nc.psum_tensor(f"acc{m}", [128, 512], mybir.dt.float32))
            for m in range(MT)]
        mn = [ctx.enter_context(
            nc.sbuf_tensor(f"mn{m}", [128, OSH], mybir.dt.float32))
            for m in range(MT)]
        ex = [ctx.enter_context(
            nc.sbuf_tensor(f"ex{m}", [128, OSH], mybir.dt.float32))
            for m in range(MT)]
        rl = [ctx.enter_context(
            nc.sbuf_tensor(f"rl{m}", [128, OSH], mybir.dt.float32))
            for m in range(MT)]
        oo = [ctx.enter_context(
            nc.sbuf_tensor(f"oo{m}", [128, OSH], mybir.dt.float32))
            for m in range(MT)]
        lnb = ctx.enter_context(
            nc.sbuf_tensor("lnb", [128, 1], mybir.dt.float32))

        xsems = [ctx.enter_context(nc.semaphore(name=f"xsem{g}"))
                 for g in range(NG)]
        wsems = [ctx.enter_context(nc.semaphore(name=f"wsem{g}"))
                 for g in range(NG)]
        pe_sem = ctx.enter_context(nc.semaphore(name="pe_sem"))
        dve_sem = ctx.enter_context(nc.semaphore(name="dve_sem"))
        act_sem = ctx.enter_context(nc.semaphore(name="act_sem"))
        o_sem = ctx.enter_context(nc.semaphore(name="o_sem"))
        od_sem = ctx.enter_context(nc.semaphore(name="od_sem"))
        block = ctx.enter_context(nc.Block())

        @block.sync
        def _(sync):
            for g, (a, b) in enumerate(bounds):
                sync.dma_start(
                    xt_sb[:, a * BSH:b * BSH],
                    xt_d[:, a * BSH:b * BSH]).then_inc(xsems[g], 16)
            for m in range(MT):
                sync.wait_ge(o_sem, m + 1)
                sync.dma_start(
                    out_d[m * 128:(m + 1) * 128, :],
                    oo[m][:]).then_inc(od_sem, 16)
            sync.wait_ge(od_sem, 32)

        # NOTE: the ScalarEngine must not read PSUM on this runner (hard
        # device fault, found by bisection) — all PSUM reads go via DVE.
        @block.scalar
        def _(scalar):
            for g, (a, b) in enumerate(bounds):
                scalar.dma_start(
                    w_sb[:, a * OSH:b * OSH],
                    w_d[:, a * OSH:b * OSH]).then_inc(wsems[g], 16)
            for m in range(MT):
                scalar.wait_ge(dve_sem, m + 2)  # memset(1) + mn[m]
                scalar.activation(ex[m][:], mn[m][:],
                                  mybir.ActivationFunctionType.Exp,
                                  bias=lnb[:]).then_inc(act_sem, 1)

        @block.vector
        def _(vector):
            vector.memset(lnb[:], math.log(lam_al)).then_inc(dve_sem, 1)
            for m in range(MT):
                vector.wait_ge(pe_sem, m + 1)
                # rl = max(lam*acc, 0) == lam*relu(acc); consumed only by
                # this engine's own stt below -> no semaphore needed
                vector.tensor_scalar(rl[m][:], accs[m][:, :OSH],
                                     SELU_SCALE, 0.0,
                                     mybir.AluOpType.mult,
                                     mybir.AluOpType.max)
                vector.tensor_scalar_min(
                    mn[m][:], accs[m][:, :OSH], 0.0).then_inc(dve_sem, 1)
                vector.wait_ge(act_sem, m + 1)
                vector.scalar_tensor_tensor(
                    oo[m][:], ex[m][:], -lam_al, rl[m][:],
                    mybir.AluOpType.add,
                    mybir.AluOpType.add).then_inc(o_sem, 1)

        @block.tensor
        def _(tensor):
            def mm(k, m):
                return tensor.matmul(
                    accs[m][:, :OSH],
                    xt_sb[:, k * BSH + m * 128: k * BSH + (m + 1) * 128],
                    w_sb[:, k * OSH:(k + 1) * OSH],
                    start=(k == 0),
                    stop=(k == KT - 1),
                )

            for g, (a, b) in enumerate(bounds):
                tensor.wait_ge(xsems[g], 16)
                tensor.wait_ge(wsems[g], 16)
                for k in range(a, min(b, KT - TAIL_M)):
                    for m in range(MT):
                        mm(k, m)
            for m in range(MT):
                for k in range(KT - TAIL_M, KT):
                    inst = mm(k, m)
                inst.then_inc(pe_sem, 1)

    nc.compile()
    return nc


def _prepare_in_maps(x, weight, bias, out_idx, in_idx):
    x = np.asarray(x, dtype=np.float32)
    weight = np.asarray(weight, dtype=np.float32)
    bias = np.asarray(bias, dtype=np.float32)
    oi = np.asarray(out_idx).astype(np.int64)
    ii = np.asarray(in_idx).astype(np.int64)

    # densify the edge list; duplicate (i, o) pairs accumulate
    W = np.bincount(ii * OUT_F + oi, weights=weight.astype(np.float64),
                    minlength=IN_F * OUT_F).astype(np.float32)
    W = W.reshape(IN_F, OUT_F)

    Wp = np.zeros((KPAD, OUT_F), dtype=np.float32)
    Wp[:IN_F] = W
    Wp[IN_F] = bias           # bias row, matched by the ones row in x^T
    xtp = np.zeros((KPAD, B), dtype=np.float32)
    xtp[:IN_F] = x.T
    xtp[IN_F] = 1.0

    w_bf = Wp.astype(ml_dtypes.bfloat16)
    xt_bf = xtp.astype(ml_dtypes.bfloat16)

    in_maps = []
    for c in range(NCORES):
        b, o = divmod(c, OS)
        xt_shard = np.ascontiguousarray(xt_bf[:, b * BSH:(b + 1) * BSH])
        w_shard = np.ascontiguousarray(w_bf[:, o * OSH:(o + 1) * OSH])
        in_maps.append({
            # -> partition-major [128, KT*cols]
            "xt": np.ascontiguousarray(
                xt_shard.reshape(KT, 128, BSH).transpose(1, 0, 2)
            ).reshape(128, KT * BSH),
            "w": np.ascontiguousarray(
                w_shard.reshape(KT, 128, OSH).transpose(1, 0, 2)
            ).reshape(128, KT * OSH),
        })
    return in_maps


def _assemble(results):
    y = np.empty((B, OUT_F), dtype=np.float32)
    for c in range(NCORES):
        b, o = divmod(c, OS)
        y[b * BSH:(b + 1) * BSH, o * OSH:(o + 1) * OSH] = \
            np.asarray(results[c]["out"]).astype(np.float32)
    return y


USE_RAW = False  # hand-synchronized build vs TileContext build


def get_compiled():
    global _compiled
    if _compiled is None:
        import os
        raw = USE_RAW if os.environ.get("KERNEL_RAW") is None \
            else os.environ["KERNEL_RAW"] == "1"
        _compiled = _build_raw() if raw else _build()
    return _compiled


def kernel(x, weight, bias, out_idx, in_idx):
    in_maps = _prepare_in_maps(x, weight, bias, out_idx, in_idx)
    nc = get_compiled()
    last_err = None
    for _attempt in range(3):  # retry transient device/runtime hiccups
        try:
            res = run_bass_kernel_spmd(nc, in_maps,
                                       core_ids=list(range(NCORES)))
            return _assemble(res.results)
        except Exception as e:  # noqa: BLE001
            last_err = e
    raise last_err


# revision 35
# speedup vs baseline: 1.1189x; 1.0046x over previous
"""Trainium2 kernel for SparseLinear + bias + SELU (nn_AEEncoder).

Reference computation:
    y[b, o] = selu( sum_{e: out_idx[e]==o} weight[e] * x[b, in_idx[e]] + bias[o] )
with B=512, IN_F=20000, OUT_F=1000, NNZ=500000.

Strategy
--------
The edge list arrives as concrete numpy arrays at call time, so the sparse
weights are densified on the host into W[IN_F, OUT_F] (duplicate edges
accumulate). The device kernel is then a dense matmul y = x @ W + bias
followed by SELU, executed in bf16 (f32 PSUM accumulation).

Sharding: a 2-way batch x 4-way output-column grid over the 8 NeuronCores.
Each core computes a full [256, 250] block of the output independently —
no cross-core collectives (collectives pay a large all-core sync barrier
under this runner). The bias is folded into the matmul as one extra
contraction row (x^T gets a row of ones, W gets the bias row), so the
on-chip epilogue is only the SELU.

Per-core: x^T shard [20096, 256] bf16 + W shard [20096, 250] bf16 are
DMA-streamed to SBUF in k-tile groups (partition-major DRAM layout so
every DMA is long contiguous runs); 314 accumulating matmuls
(2 M-tiles x 157 K-tiles, N=250) run concurrently with the DMA stream;
SELU is computed straight out of PSUM and the [256, 250] f32 block is
DMA'd out. The host assembles the 2x4 grid into the full [512, 1000].
"""

import numpy as np
import ml_dtypes

import concourse.bass as bass
import concourse.mybir as mybir
import concourse.tile as tile
from concourse import bacc
from concourse.bass_utils import run_bass_kernel_spmd

B, IN_F, OUT_F = 512, 20000, 1000
NCORES = 8
BS, OS = 2, 4          # batch split x out-column split (BS*OS == NCORES)
BSH = B // BS          # 256 batch rows per core
OSH = OUT_F // OS      # 250 output columns per core
KPAD = 20096           # padded contraction dim: 157 k-tiles of 128
KT = KPAD // 128       # 157 k-tiles (row IN_F==20000 carries the bias)
# k-tiles per DMA group: moderate first group -> PE starts soon; ramp-down
# -> short compute tail after the final transfer. Sums to KT.
GROUPS = [6, 14, 20, 20, 20, 20, 20, 17, 10, 6, 4]
TAIL_M = 16            # last k-tiles emitted m-major so epilogue(m=0)
                       # overlaps the final matmuls of m=1
MT = BSH // 128        # 2 M-tiles per core
NCHUNK = 2             # epilogue column chunks
OUT_SPLIT_RING = False  # m=1 output DMA on the scalar ring (parallel receipts)

SELU_SCALE = 1.0507009873554805
SELU_ALPHA = 1.6732632423543772

_compiled = None


def _build():
    nc = bacc.Bacc("TRN2", target_bir_lowering=False, debug=False,
                   num_devices=NCORES)
    # partition-major layouts: row p holds that partition's whole k-stream,
    # so each DMA is 128 fully-contiguous runs (no small-packet penalty)
    xt_d = nc.dram_tensor("xt", [128, KT * BSH], mybir.dt.bfloat16,
                          kind="ExternalInput")
    w_d = nc.dram_tensor("w", [128, KT * OSH], mybir.dt.bfloat16,
                         kind="ExternalInput")
    # device emits bf16; the host upcasts to float32 (error budget allows it,
    # halves the output traffic and speeds the final DVE writes)
    out_d = nc.dram_tensor("out", [BSH, OSH], mybir.dt.bfloat16,
                           kind="ExternalOutput")

    with tile.TileContext(nc) as tc:
        with (
            tc.tile_pool(name="sb", bufs=1) as sb,
            tc.tile_pool(name="ps", bufs=1, space="PSUM") as ps,
        ):
            xt_sb = sb.tile([128, KT * BSH], mybir.dt.bfloat16)
            w_sb = sb.tile([128, KT * OSH], mybir.dt.bfloat16)
            assert sum(GROUPS) == KT
            g0 = 0
            for gsz in GROUPS:
                g1 = g0 + gsz
                nc.sync.dma_start(
                    xt_sb[:, g0 * BSH:g1 * BSH],
                    xt_d[:, g0 * BSH:g1 * BSH],
                )
                nc.scalar.dma_start(
                    w_sb[:, g0 * OSH:g1 * OSH],
                    w_d[:, g0 * OSH:g1 * OSH],
                )
                g0 = g1

            accs = [
                ps.tile([128, OSH], mybir.dt.float32,
                        name=f"acc{m}", tag=f"acc{m}")
                for m in range(MT)
            ]
            def mm(k, m):
                nc.tensor.matmul(
                    accs[m][:],
                    xt_sb[:, k * BSH + m * 128: k * BSH + (m + 1) * 128],
                    w_sb[:, k * OSH:(k + 1) * OSH],
                    start=(k == 0),
                    stop=(k == KT - 1),
                )

            for k in range(KT - TAIL_M):
                for m in range(MT):
                    mm(k, m)
            for m in range(MT):
                for k in range(KT - TAIL_M, KT):
                    mm(k, m)

            import math
            lam_al = SELU_SCALE * SELU_ALPHA
            lnb = sb.tile([128, 1], mybir.dt.float32)
            nc.vector.memset(lnb[:], math.log(lam_al))
            CW = OSH // NCHUNK  # 62/63-col chunks: DVE/ACT chains pipeline
            for m in range(MT):
                # selu(v) = lam*relu(v) + lam*al*(exp(min(v,0)) - 1)
                #         = (lam*al*exp(min(v,0)) - lam*al) + lam*relu(v)
                rl = sb.tile([128, OSH], mybir.dt.float32,
                             name=f"rl{m}", tag=f"rl{m}")
                mn = sb.tile([128, OSH], mybir.dt.float32,
                             name=f"mn{m}", tag=f"mn{m}")
                ex = sb.tile([128, OSH], mybir.dt.float32,
                             name=f"ex{m}", tag=f"ex{m}")
                oo = sb.tile([128, OSH], mybir.dt.bfloat16,
                             name=f"oo{m}", tag=f"oo{m}")
                for h in range(NCHUNK):
                    cs = slice(h * CW, (h + 1) * CW if h < NCHUNK - 1
                               else OSH)
                    nc.scalar.activation(rl[:, cs], accs[m][:, cs],
                                         mybir.ActivationFunctionType.Relu,
                                         scale=SELU_SCALE)
                    nc.vector.tensor_scalar_min(mn[:, cs], accs[m][:, cs],
                                                0.0)
                    # exp(mn + ln(lam*al)) == lam*al*exp(mn)
                    nc.scalar.activation(ex[:, cs], mn[:, cs],
                                         mybir.ActivationFunctionType.Exp,
                                         bias=lnb[:])
                    nc.vector.scalar_tensor_tensor(
                        oo[:, cs], ex[:, cs], -lam_al, rl[:, cs],
                        mybir.AluOpType.add, mybir.AluOpType.add)
                eng = nc.scalar if (OUT_SPLIT_RING and m == 1) else nc.sync
                eng.dma_start(out_d[m * 128:(m + 1) * 128, :], oo[:])

    nc.compile()
    return nc


def _build_raw():
    """Hand-synchronized build (no TileContext): cuts Tile's entry/exit
    barriers and starts the DMA streams immediately."""
    import math
    from contextlib import ExitStack

    lam_al = SELU_SCALE * SELU_ALPHA
    nc = bacc.Bacc("TRN2", target_bir_lowering=False, debug=False,
                   num_devices=NCORES)
    xt_d = nc.dram_tensor("xt", [128, KT * BSH], mybir.dt.bfloat16,
                          kind="ExternalInput")
    w_d = nc.dram_tensor("w", [128, KT * OSH], mybir.dt.bfloat16,
                         kind="ExternalInput")
    out_d = nc.dram_tensor("out", [BSH, OSH], mybir.dt.float32,
                           kind="ExternalOutput")

    bounds = []
    g0 = 0
    for gsz in GROUPS:
        bounds.append((g0, g0 + gsz))
        g0 += gsz
    assert g0 == KT
    NG = len(bounds)

    with ExitStack() as ctx:
        xt_sb = ctx.enter_context(
            nc.sbuf_tensor("xt_sb", [128, KT * BSH], mybir.dt.bfloat16))
        w_sb = ctx.enter_context(
            nc.sbuf_tensor("w_sb", [128, KT * OSH], mybir.dt.bfloat16))
        # one PSUM bank per accumulator (512 f32 = full bank) so the two
        # accumulators never share a bank (PE-write/DVE-read hazard)
        accs = [ctx.enter_context(
            nc.psum_tensor(f"acc{m}", [128, 512], mybir.dt.float32))
            for m in range(MT)]
        mn = [ctx.enter_context(
            nc.sbuf_tensor(f"mn{m}", [128, OSH], mybir.dt.float32))
            for m in range(MT)]
        ex = [ctx.enter_context(
            nc.sbuf_tensor(f"ex{m}", [128, OSH], mybir.dt.float32))
            for m in range(MT)]
        rl = [ctx.enter_context(
            nc.sbuf_tensor(f"rl{m}", [128, OSH], mybir.dt.float32))
            for m in range(MT)]
        oo = [ctx.enter_context(
            nc.sbuf_tensor(f"oo{m}", [128, OSH], mybir.dt.float32))
            for m in range(MT)]
        lnb = ctx.enter_context(
            nc.sbuf_tensor("lnb", [128, 1], mybir.dt.float32))

        xsems = [ctx.enter_context(nc.semaphore(name=f"xsem{g}"))
                 for g in range(NG)]
        wsems = [ctx.enter_context(nc.semaphore(name=f"wsem{g}"))
                 for g in range(NG)]
        pe_sem = ctx.enter_context(nc.semaphore(name="pe_sem"))
        dve_sem = ctx.enter_context(nc.semaphore(name="dve_sem"))
        act_sem = ctx.enter_context(nc.semaphore(name="act_sem"))
        o_sem = ctx.enter_context(nc.semaphore(name="o_sem"))
        od_sem = ctx.enter_context(nc.semaphore(name="od_sem"))
        block = ctx.enter_context(nc.Block())

        @block.sync
        def _(sync):
            for g, (a, b) in enumerate(bounds):
                sync.dma_start(
                    xt_sb[:, a * BSH:b * BSH],
                    xt_d[:, a * BSH:b * BSH]).then_inc(xsems[g], 16)
            for m in range(MT):
                sync.wait_ge(o_sem, m + 1)
                sync.dma_start(
                    out_d[m * 128:(m + 1) * 128, :],
                    oo[m][:]).then_inc(od_sem, 16)
            sync.wait_ge(od_sem, 32)

        # NOTE: the ScalarEngine must not read PSUM on this runner (hard
        # device fault, found by bisection) — all PSUM reads go via DVE.
        @block.scalar
        def _(scalar):
            for g, (a, b) in enumerate(bounds):
                scalar.dma_start(
                    w_sb[:, a * OSH:b * OSH],
                    w_d[:, a * OSH:b * OSH]).then_inc(wsems[g], 16)
            for m in range(MT):
                scalar.wait_ge(dve_sem, m + 2)  # memset(1) + mn[m]
                scalar.activation(ex[m][:], mn[m][:],
                                  mybir.ActivationFunctionType.Exp,
                                  bias=lnb[:]).then_inc(act_sem, 1)

        @block.vector
        def _(vector):
            vector.memset(lnb[:], math.log(lam_al)).then_inc(dve_sem, 1)
            for m in range(MT):
                vector.wait_ge(pe_sem, m + 1)
                # rl = max(lam*acc, 0) == lam*relu(acc); consumed only by
                # this engine's own stt below -> no semaphore needed
                vector.tensor_scalar(rl[m][:], accs[m][:, :OSH],
                                     SELU_SCALE, 0.0,
                                     mybir.AluOpType.mult,
                                     mybir.AluOpType.max)
                vector.tensor_scalar_min(
                    mn[m][:], accs[m][:, :OSH], 0.0).then_inc(dve_sem, 1)
                vector.wait_ge(act_sem, m + 1)
                vector.scalar_tensor_tensor(
                    oo[m][:], ex[m][:], -lam_al, rl[m][:],
                    mybir.AluOpType.add,
                    mybir.AluOpType.add).then_inc(o_sem, 1)

        @block.tensor
        def _(tensor):
            def mm(k, m):
                return tensor.matmul(
                    accs[m][:, :OSH],
                    xt_sb[:, k * BSH + m * 128: k * BSH + (m + 1) * 128],
                    w_sb[:, k * OSH:(k + 1) * OSH],
                    start=(k == 0),
                    stop=(k == KT - 1),
                )

            for g, (a, b) in enumerate(bounds):
                tensor.wait_ge(xsems[g], 16)
                tensor.wait_ge(wsems[g], 16)
                for k in range(a, min(b, KT - TAIL_M)):
                    for m in range(MT):
                        mm(k, m)
            for m in range(MT):
                for k in range(KT - TAIL_M, KT):
                    inst = mm(k, m)
                inst.then_inc(pe_sem, 1)

    nc.compile()
    return nc


def _prepare_in_maps(x, weight, bias, out_idx, in_idx):
    x = np.asarray(x, dtype=np.float32)
    weight = np.asarray(weight, dtype=np.float32)
    bias = np.asarray(bias, dtype=np.float32)
    oi = np.asarray(out_idx).astype(np.int64)
    ii = np.asarray(in_idx).astype(np.int64)

    # densify the edge list; duplicate (i, o) pairs accumulate
    W = np.bincount(ii * OUT_F + oi, weights=weight.astype(np.float64),
                    minlength=IN_F * OUT_F).astype(np.float32)
    W = W.reshape(IN_F, OUT_F)

    Wp = np.zeros((KPAD, OUT_F), dtype=np.float32)
    Wp[:IN_F] = W
    Wp[IN_F] = bias           # bias row, matched by the ones row in x^T
    xtp = np.zeros((KPAD, B), dtype=np.float32)
    xtp[:IN_F] = x.T
    xtp[IN_F] = 1.0

    w_bf = Wp.astype(ml_dtypes.bfloat16)
    xt_bf = xtp.astype(ml_dtypes.bfloat16)

    in_maps = []
    for c in range(NCORES):
        b, o = divmod(c, OS)
        xt_shard = np.ascontiguousarray(xt_bf[:, b * BSH:(b + 1) * BSH])
        w_shard = np.ascontiguousarray(w_bf[:, o * OSH:(o + 1) * OSH])
        in_maps.append({
            # -> partition-major [128, KT*cols]
            "xt": np.ascontiguousarray(
                xt_shard.reshape(KT, 128, BSH).transpose(1, 0, 2)
            ).reshape(128, KT * BSH),
            "w": np.ascontiguousarray(
                w_shard.reshape(KT, 128, OSH).transpose(1, 0, 2)
            ).reshape(128, KT * OSH),
        })
    return in_maps


def _assemble(results):
    y = np.empty((B, OUT_F), dtype=np.float32)
    for c in range(NCORES):
        b, o = divmod(c, OS)
        y[b * BSH:(b + 1) * BSH, o * OSH:(o + 1) * OSH] = \
            np.asarray(results[c]["out"]).astype(np.float32)
    return y


USE_RAW = False  # hand-synchronized build vs TileContext build


def get_compiled():
    global _compiled
    if _compiled is None:
        import os
        raw = USE_RAW if os.environ.get("KERNEL_RAW") is None \
            else os.environ["KERNEL_RAW"] == "1"
        _compiled = _build_raw() if raw else _build()
    return _compiled


def kernel(x, weight, bias, out_idx, in_idx):
    in_maps = _prepare_in_maps(x, weight, bias, out_idx, in_idx)
    nc = get_compiled()
    last_err = None
    for _attempt in range(3):  # retry transient device/runtime hiccups
        try:
            res = run_bass_kernel_spmd(nc, in_maps,
                                       core_ids=list(range(NCORES)))
            return _assemble(res.results)
        except Exception as e:  # noqa: BLE001
            last_err = e
    raise last_err
